# revision 5
# baseline (speedup 1.0000x reference)
"""Trainium2 8-core attention kernel (v4, q-major).

Problem: B=2, H=16, S=2048, D=64 dense attention, f32 I/O.
Sharding: B*H = 32 head-batches -> 4 heads per NeuronCore (embarrassingly
parallel, no collectives).

v4 restructures v3 into q-major columns:
  For each head h, for each q-column qc (512 wide):
    - 16 QK matmuls (one per k-tile, row-tiled pairs) write scores into a
      6-bank PSUM ring (s_arena [128, 6, 512] f32; bank = kt % 6).
    - exp is issued in multi-bank segments: ScalarE ACT gets FD=2048/1536
      instructions (amortizes the 352-cycle ACTIVATE overhead), VectorE
      Schraudolph (f32 -> round(A*s+B) int16, bitcast bf16) gets the rest.
    - PV for column qc-1 (same head; 16 accumulating matmuls into
      o_ps [65, 512], V'=[V|ones] so row 64 = softmax denominator) is
      interleaved into the PE FIFO between QK bursts.
    - epilogue: DVE reciprocal_approx_fast directly from PSUM row 64,
      gpsimd partition-broadcast, DVE multiply, DMA out. No PSUM->SBUF
      den copy (v3 paid one).
  PV lags exp by ONE column (v3: a full head), so the kernel tail is just
  the last column's PV chase + epilogue instead of 2 full PV chunks.

Host side only reshapes/transposes/casts (unchanged from v3):
  qt, kt: [4, 128, 2048] bf16 (d on partitions, rows 64:128 duplicate 0:64)
  vp:     [4, 128, 16, 65] bf16 (k%128 on partitions, ones column appended)
  ot:     [4, 64, 2048] f32 (transposed; host transposes back)
"""

import numpy as np
import ml_dtypes

import concourse.bass as bass
import concourse.tile as tile
from concourse import bacc, mybir
from concourse.bass_utils import run_bass_kernel_spmd

B, H, S, D = 2, 16, 2048, 64
NCORES = 8
HPC = (B * H) // NCORES  # heads per core = 4
P = 128
KT = S // P  # 16 k-tiles
QC = 4  # q columns per head (512 wide)
QW = S // QC  # 512
NBANK = 6  # PSUM score ring banks
SCALE = 1.0 / np.sqrt(D)  # 0.125

SCH_A = float(P * np.log2(np.e) * SCALE)
SCH_B = float(P * 127 - 7.5)

# Per-column exp segments: (engine, kt_start, n_kt). Banks kt%6 must not
# wrap inside a segment. ScalarE ~62.5% of tiles, big FDs first.
SEGS = [
    ("s", 0, 4),   # ACT FD=2048
    ("d", 4, 2),   # DVE FD=1024
    ("s", 6, 3),   # ACT FD=1536
    ("d", 9, 3),   # DVE FD=1536
    ("s", 12, 3),  # ACT FD=1536
    ("d", 15, 1),  # DVE FD=512
]
# PE FIFO interleave: after the QK tiles of segment i are emitted, run
# this many PV matmuls of the previous column's chunk.
PV_AFTER_SEG = [0, 9, 3, 2, 2, 0]

WARMUP_MM = 18

f32 = mybir.dt.float32
bf16 = mybir.dt.bfloat16
i16 = mybir.dt.int16


class PVChunk:
    """One output chunk (h, qc): 16 accumulating PV matmuls + epilogue."""

    def __init__(self, h, qc, p_col, v_b):
        self.h, self.qc, self.p_col, self.v_b = h, qc, p_col, v_b
        self.o_ps = None
        self.k = 0

    def step(self, nc, pools, aps, n_mm):
        qt, kt, vp, ot = aps
        _, _, _, epi_pool, _, ps_o = pools
        if self.k >= KT:
            return
        if self.o_ps is None:
            self.o_ps = ps_o.tile([P, QW], f32, tag="o")
        for _ in range(n_mm):
            if self.k >= KT:
                break
            nc.tensor.matmul(
                self.o_ps[: D + 1, :],
                lhsT=self.v_b[:, self.k, :],
                rhs=self.p_col[:, self.k * QW : (self.k + 1) * QW],
                start=(self.k == 0),
                stop=(self.k == KT - 1),
                skip_group_check=True,
            )
            self.k += 1
        if self.k >= KT:
            self.finish(nc, pools, aps)

    def finish(self, nc, pools, aps):
        qt, kt, vp, ot = aps
        _, _, _, epi_pool, _, _ = pools
        o_ps = self.o_ps
        rden = epi_pool.tile([1, QW], f32, tag="rden")
        nc.vector.reciprocal_approx_fast(rden[:], o_ps[D : D + 1, :])
        rbc = epi_pool.tile([D, QW], f32, tag="rbc")
        nc.gpsimd.partition_broadcast(rbc[:], rden[0:1, :])
        ot_sb = epi_pool.tile([D, QW], f32, tag="ot")
        nc.vector.tensor_mul(ot_sb[:], o_ps[:D, :], rbc[:])
        nc.sync.dma_start(
            ot[self.h, :, self.qc * QW : (self.qc + 1) * QW], ot_sb[:]
        )
        self.k = KT + 1


def emit_loads(nc, pools, aps, h):
    """DMA head h's inputs, split across queues; small first chunks."""
    qt, kt, vp, ot = aps
    qk_pool, v_pool, _, _, _, _ = pools
    qt_b = qk_pool.tile([P, S], bf16, tag="qt")
    kt_b = qk_pool.tile([P, S], bf16, tag="kt")
    if h == 0:
        # First column needs kt[:, :] progressively + qt[:, 0:512] only.
        nc.sync.dma_start(kt_b[:, 0:512], kt[h, :, 0:512])
        nc.scalar.dma_start(qt_b[:, 0:512], qt[h, :, 0:512])
        nc.gpsimd.dma_start(kt_b[:, 512:1024], kt[h, :, 512:1024])
        nc.scalar.dma_start(qt_b[:, 512:1024], qt[h, :, 512:1024])
        nc.sync.dma_start(kt_b[:, 1024:1536], kt[h, :, 1024:1536])
        nc.gpsimd.dma_start(kt_b[:, 1536:2048], kt[h, :, 1536:2048])
        nc.scalar.dma_start(qt_b[:, 1024:2048], qt[h, :, 1024:2048])
    else:
        nc.sync.dma_start(kt_b[:, 0 : S // 2], kt[h, :, 0 : S // 2])
        nc.gpsimd.dma_start(kt_b[:, S // 2 :], kt[h, :, S // 2 :])
        nc.sync.dma_start(qt_b[:, 0 : S // 2], qt[h, :, 0 : S // 2])
        nc.gpsimd.dma_start(qt_b[:, S // 2 :], qt[h, :, S // 2 :])
    v_b = v_pool.tile([P, KT, D + 1], bf16, tag="v")
    nc.sync.dma_start(v_b[:], vp[h])
    return qt_b, kt_b, v_b


def emit_qk(nc, s_arena, qt_b, kt_b, kt_i, qc):
    """One QK matmul: scores for (k-tile kt_i, q column qc) -> ring bank."""
    bank = kt_i % NBANK
    r0 = 0 if (kt_i % 2 == 0) else 64
    nc.tensor.matmul(
        s_arena[:, bank * QW : (bank + 1) * QW],
        lhsT=kt_b[r0 : r0 + 64, kt_i * P : (kt_i + 1) * P],
        rhs=qt_b[r0 : r0 + 64, qc * QW : (qc + 1) * QW],
        start=True,
        stop=True,
        tile_position=(r0, 0),
    )


def emit_exp_seg(nc, s_arena, p_col, eng, kt0, n):
    bank = kt0 % NBANK
    src = s_arena[:, bank * QW : (bank + n) * QW]
    dst = p_col[:, kt0 * QW : (kt0 + n) * QW]
    if eng == "s":
        nc.scalar.activation(
            dst, src, mybir.ActivationFunctionType.Exp, scale=float(SCALE)
        )
    else:
        nc.vector.tensor_scalar(
            dst.bitcast(i16),
            src,
            SCH_A,
            SCH_B,
            mybir.AluOpType.mult,
            mybir.AluOpType.add,
        )


def build_nc():
    nc = bacc.Bacc("TRN2", target_bir_lowering=False, debug=False)
    qt = nc.dram_tensor("qt", [HPC, P, S], bf16, kind="ExternalInput").ap()
    kt = nc.dram_tensor("kt", [HPC, P, S], bf16, kind="ExternalInput").ap()
    vp = nc.dram_tensor("vp", [HPC, P, KT, D + 1], bf16, kind="ExternalInput").ap()
    ot = nc.dram_tensor("ot", [HPC, D, S], f32, kind="ExternalOutput").ap()
    aps = (qt, kt, vp, ot)

    with tile.TileContext(nc) as tc:
        with (
            tc.tile_pool(name="qk", bufs=2) as qk_pool,
            tc.tile_pool(name="v", bufs=2) as v_pool,
            tc.tile_pool(name="p", bufs=3) as p_pool,
            tc.tile_pool(name="epi", bufs=3) as epi_pool,
            tc.tile_pool(name="ps_s", bufs=1, space="PSUM") as ps_s,
            tc.tile_pool(name="ps_o", bufs=2, space="PSUM") as ps_o,
        ):
            pools = (qk_pool, v_pool, p_pool, epi_pool, ps_s, ps_o)

            # 6-bank score ring, allocated once.
            s_arena = ps_s.tile([P, NBANK * QW], f32, tag="s")

            # PE warmup (HAM un-throttle) while input DMA lands.
            warm_w = qk_pool.tile([P, P], bf16, tag="warm")
            nc.gpsimd.memset(warm_w[:], 0.0)
            warm_ps = ps_o.tile([P, QW], f32, tag="o")
            for _ in range(WARMUP_MM):
                nc.tensor.matmul(
                    warm_ps[:, :P], lhsT=warm_w[:], rhs=warm_w[:],
                    start=True, stop=True,
                )

            prev = None  # PVChunk still accumulating
            epi_q = []  # PV-complete chunks awaiting epilogue emission
            for h in range(HPC):
                qt_b, kt_b, v_b = emit_loads(nc, pools, aps, h)
                for qc in range(QC):
                    p_col = p_pool.tile([P, KT * QW], bf16, tag="p")
                    last_col = h == HPC - 1 and qc == QC - 1
                    for si, (eng, kt0, n) in enumerate(SEGS):
                        for kt_i in range(kt0, kt0 + n):
                            emit_qk(nc, s_arena, qt_b, kt_b, kt_i, qc)
                        emit_exp_seg(nc, s_arena, p_col, eng, kt0, n)
                        if si == 2 and epi_q:
                            epi_q[0].epi_a(nc, pools)
                        if si == 4 and epi_q:
                            epi_q.pop(0).epi_b(nc, pools, aps)
                        if prev is not None and PV_AFTER_SEG[si]:
                            prev.step(nc, pools, aps, PV_AFTER_SEG[si])
                    if prev is not None:
                        prev.step(nc, pools, aps, KT)  # any remainder
                        assert prev.done()
                        epi_q.append(prev)
                    prev = PVChunk(h, qc, p_col, v_b)
                    if last_col:
                        prev.step(nc, pools, aps, KT)
                        epi_q.append(prev)
            # drain remaining epilogues (last two chunks)
            for c in epi_q:
                c.epi_a(nc, pools)
            for c in epi_q:
                c.epi_b(nc, pools, aps)

    nc.compile()
    return nc


def shard_inputs(Q, K, V):
    Qh = np.asarray(Q, dtype=np.float32).reshape(B * H, S, D)
    Kh = np.asarray(K, dtype=np.float32).reshape(B * H, S, D)
    Vh = np.asarray(V, dtype=np.float32).reshape(B * H, S, D)

    in_maps = []
    for c in range(NCORES):
        sl = slice(c * HPC, (c + 1) * HPC)
        qt = np.empty((HPC, P, S), dtype=ml_dtypes.bfloat16)
        kt = np.empty((HPC, P, S), dtype=ml_dtypes.bfloat16)
        qt[:, :D, :] = Qh[sl].transpose(0, 2, 1).astype(ml_dtypes.bfloat16)
        kt[:, :D, :] = Kh[sl].transpose(0, 2, 1).astype(ml_dtypes.bfloat16)
        qt[:, D:, :] = qt[:, :D, :]
        kt[:, D:, :] = kt[:, :D, :]
        vp = np.ones((HPC, S, D + 1), dtype=np.float32)
        vp[:, :, :D] = Vh[sl]
        vp = (
            vp.reshape(HPC, KT, P, D + 1)
            .transpose(0, 2, 1, 3)
            .astype(ml_dtypes.bfloat16)
        )
        in_maps.append({"qt": np.ascontiguousarray(qt),
                        "kt": np.ascontiguousarray(kt),
                        "vp": np.ascontiguousarray(vp)})
    return in_maps


_NC_CACHE = None


def kernel(Q, K, V):
    global _NC_CACHE
    if _NC_CACHE is None:
        _NC_CACHE = build_nc()
    nc = _NC_CACHE
    in_maps = shard_inputs(Q, K, V)
    res = run_bass_kernel_spmd(nc, in_maps, core_ids=list(range(NCORES)))
    out = np.empty((B * H, S, D), dtype=np.float32)
    for c in range(NCORES):
        out[c * HPC : (c + 1) * HPC] = res.results[c]["ot"].transpose(0, 2, 1)
    return out.reshape(B, H, S, D)


if __name__ == "__main__":
    nc = build_nc()
    print("compiled OK")


# revision 6
# speedup vs baseline: 1.4221x; 1.4221x over previous
"""Trainium2 8-core attention kernel (v5, q-major 1024-wide columns).

Problem: B=2, H=16, S=2048, D=64 dense attention, f32 I/O.
Sharding: B*H = 32 head-batches -> 4 heads per NeuronCore (embarrassingly
parallel, no collectives).

Structure (v3's proven PSUM geometry + q-major scheduling + wide ACTs):
  For each head h, for each 1024-wide q column (2 per head):
    - per k-tile kt (16): one row-tiled QK matmul pair (rows 0:64 compute
      q-subhalf 0, duplicated rows 64:128 compute subhalf 1, concurrent)
      writes scores [128, 1024] f32 into slot kt%3 of a 3-slot PSUM arena
      (6 banks total; o_ps chunks use the remaining 2).
    - exp: ScalarE ACT covers kt PAIRS (slots 0-1, FD=2048 -> amortizes the
      352-cycle ACTIVATE overhead; 10/16 tiles), VectorE Schraudolph
      (f32 -> round(A*s+B) int16 bitcast bf16) takes slot-2 tiles + kt15
      (6/16).
    - PV for the previous column's two 512-wide chunks (V'=[V|ones], row 64
      = softmax denominator) is woven between QK pairs: 32 accumulating
      matmuls keep the PE dense (HAM stays warm).
    - epilogue per chunk, phase-split so the DVE FIFO never blocks:
      A = den copy + reciprocal + gpsimd partition-broadcast kick,
      B = multiply + DMA out, emitted at points where deps are long done.
  PV lags exp by one column, so the tail is only the last column's chase.

Host side only reshapes/transposes/casts (unchanged from v3):
  qt, kt: [4, 128, 2048] bf16 (d on partitions, rows 64:128 duplicate 0:64)
  vp:     [4, 128, 16, 65] bf16 (k%128 on partitions, ones column appended)
  ot:     [4, 64, 2048] f32 (transposed; host transposes back)
"""

import numpy as np
import ml_dtypes

import concourse.bass as bass
import concourse.tile as tile
from concourse import bacc, mybir
from concourse.bass_utils import run_bass_kernel_spmd

B, H, S, D = 2, 16, 2048, 64
NCORES = 8
HPC = (B * H) // NCORES  # heads per core = 4
P = 128
KT = S // P  # 16 k-tiles
CW = 1024  # column width (q)
NCOL = S // CW  # 2 columns per head
QW = 512  # PV chunk width (q); 2 chunks per column
QC = S // QW  # 4 chunks per head (for output indexing)
NSLOT = 3  # score arena slots (each [128, 1024] f32 = 2 PSUM banks)
SCALE = 1.0 / np.sqrt(D)  # 0.125

SCH_A = float(P * np.log2(np.e) * SCALE)
SCH_B = float(P * 127 - 7.5)

# exp segments per column: (engine, kt_start, n_kt). Slot = kt % 3; pairs
# land on slots (0,1) (contiguous banks), singles on slot 2 (+ kt15 on 0).
SEGS = [
    ("s", 0, 2), ("d", 2, 1),
    ("s", 3, 2), ("d", 5, 1),
    ("s", 6, 2), ("d", 8, 1),
    ("s", 9, 2), ("d", 11, 1),
    ("s", 12, 2), ("d", 14, 1),
    ("d", 15, 1),
]
# PV matmuls of the previous column's chunks woven after each segment.
PV_AFTER = [0, 3, 3, 3, 3, 3, 3, 3, 3, 4, 4]  # sum = 32

WARMUP_MM = 18

f32 = mybir.dt.float32
bf16 = mybir.dt.bfloat16
i16 = mybir.dt.int16


class PVChunk:
    """One 512-wide output chunk: 16 accumulating PV matmuls + epilogue."""

    def __init__(self, h, qc, p_col, v_b):
        self.h, self.qc, self.p_col, self.v_b = h, qc, p_col, v_b
        self.o_ps = None
        self.k = 0
        self.sub = qc % 2  # which 512 half of the column
        self.a_done = False
        self.b_done = False

    def step(self, nc, pools, aps, n_mm):
        _, _, _, _, _, ps_o = pools
        if self.k >= KT:
            return 0
        if self.o_ps is None:
            self.o_ps = ps_o.tile([P, QW], f32, tag="o")
        took = 0
        while took < n_mm and self.k < KT:
            q0 = self.k * CW + self.sub * QW
            nc.tensor.matmul(
                self.o_ps[: D + 1, :],
                lhsT=self.v_b[:, self.k, :],
                rhs=self.p_col[:, q0 : q0 + QW],
                start=(self.k == 0),
                stop=(self.k == KT - 1),
                skip_group_check=True,
            )
            self.k += 1
            took += 1
        return took

    def done(self):
        return self.k >= KT

    def epi_a(self, nc, pools):
        _, _, _, epi_pool, _, _ = pools
        den = epi_pool.tile([1, QW], f32, tag="den")
        nc.vector.tensor_copy(den[:], self.o_ps[D : D + 1, :])
        rden = epi_pool.tile([1, QW], f32, tag="rden")
        nc.vector.reciprocal_approx_fast(rden[:], den[:])
        self.rbc = epi_pool.tile([D, QW], f32, tag="rbc")
        nc.gpsimd.partition_broadcast(self.rbc[:], rden[0:1, :])
        self.a_done = True

    def epi_b(self, nc, pools, aps):
        qt, kt, vp, ot = aps
        _, _, _, epi_pool, _, _ = pools
        ot_sb = epi_pool.tile([D, QW], f32, tag="ot")
        nc.vector.tensor_mul(ot_sb[:], self.o_ps[:D, :], self.rbc[:])
        nc.sync.dma_start(
            ot[self.h, :, self.qc * QW : (self.qc + 1) * QW], ot_sb[:]
        )
        self.b_done = True


def emit_loads(nc, pools, aps, h):
    qt, kt, vp, ot = aps
    qk_pool, v_pool, _, _, _, _ = pools
    qt_b = qk_pool.tile([P, S], bf16, tag="qt")
    kt_b = qk_pool.tile([P, S], bf16, tag="kt")
    if h == 0:
        nc.sync.dma_start(kt_b[:, 0:512], kt[h, :, 0:512])
        nc.scalar.dma_start(qt_b[:, 0:512], qt[h, :, 0:512])
        nc.gpsimd.dma_start(kt_b[:, 512:1024], kt[h, :, 512:1024])
        nc.scalar.dma_start(qt_b[:, 512:1024], qt[h, :, 512:1024])
        nc.sync.dma_start(kt_b[:, 1024:1536], kt[h, :, 1024:1536])
        nc.gpsimd.dma_start(kt_b[:, 1536:2048], kt[h, :, 1536:2048])
        nc.scalar.dma_start(qt_b[:, 1024:2048], qt[h, :, 1024:2048])
    else:
        nc.sync.dma_start(kt_b[:, 0 : S // 2], kt[h, :, 0 : S // 2])
        nc.gpsimd.dma_start(kt_b[:, S // 2 :], kt[h, :, S // 2 :])
        nc.sync.dma_start(qt_b[:, 0 : S // 2], qt[h, :, 0 : S // 2])
        nc.gpsimd.dma_start(qt_b[:, S // 2 :], qt[h, :, S // 2 :])
    v_b = v_pool.tile([P, KT, D + 1], bf16, tag="v")
    nc.sync.dma_start(v_b[:], vp[h])
    return qt_b, kt_b, v_b


def emit_qk(nc, s_arena, qt_b, kt_b, kt_i, col):
    """Row-tiled QK pair: scores [128, 1024] for (kt_i, column) -> slot."""
    slot = kt_i % NSLOT
    a0 = slot * CW
    q0 = col * CW
    nc.tensor.matmul(
        s_arena[:, a0 : a0 + QW],
        lhsT=kt_b[0:64, kt_i * P : (kt_i + 1) * P],
        rhs=qt_b[0:64, q0 : q0 + QW],
        start=True,
        stop=True,
        tile_position=(0, 0),
    )
    nc.tensor.matmul(
        s_arena[:, a0 + QW : a0 + CW],
        lhsT=kt_b[64:128, kt_i * P : (kt_i + 1) * P],
        rhs=qt_b[64:128, q0 + QW : q0 + CW],
        start=True,
        stop=True,
        tile_position=(64, 0),
    )


def emit_exp_seg(nc, s_arena, p_col, eng, kt0, n):
    slot = kt0 % NSLOT
    src = s_arena[:, slot * CW : (slot + n) * CW]
    dst = p_col[:, kt0 * CW : (kt0 + n) * CW]
    if eng == "s":
        nc.scalar.activation(
            dst, src, mybir.ActivationFunctionType.Exp, scale=float(SCALE)
        )
    else:
        nc.vector.tensor_scalar(
            dst.bitcast(i16),
            src,
            SCH_A,
            SCH_B,
            mybir.AluOpType.mult,
            mybir.AluOpType.add,
        )


def build_nc():
    nc = bacc.Bacc("TRN2", target_bir_lowering=False, debug=False)
    qt = nc.dram_tensor("qt", [HPC, P, S], bf16, kind="ExternalInput").ap()
    kt = nc.dram_tensor("kt", [HPC, P, S], bf16, kind="ExternalInput").ap()
    vp = nc.dram_tensor("vp", [HPC, P, KT, D + 1], bf16, kind="ExternalInput").ap()
    ot = nc.dram_tensor("ot", [HPC, D, S], f32, kind="ExternalOutput").ap()
    aps = (qt, kt, vp, ot)

    with tile.TileContext(nc) as tc:
        with (
            tc.tile_pool(name="qk", bufs=2) as qk_pool,
            tc.tile_pool(name="v", bufs=2) as v_pool,
            tc.tile_pool(name="p", bufs=2) as p_pool,
            tc.tile_pool(name="epi", bufs=3) as epi_pool,
            tc.tile_pool(name="ps_s", bufs=1, space="PSUM") as ps_s,
            tc.tile_pool(name="ps_o", bufs=2, space="PSUM") as ps_o,
        ):
            pools = (qk_pool, v_pool, p_pool, epi_pool, ps_s, ps_o)

            # 3-slot score arena (6 banks), allocated once.
            s_arena = ps_s.tile([P, NSLOT * CW], f32, tag="s")

            warm_w = qk_pool.tile([P, P], bf16, tag="warm")
            nc.gpsimd.memset(warm_w[:], 0.0)
            warm_ps = ps_o.tile([P, QW], f32, tag="o")
            for _ in range(WARMUP_MM):
                nc.tensor.matmul(
                    warm_ps[:, :P], lhsT=warm_w[:], rhs=warm_w[:],
                    start=True, stop=True,
                )

            prev = []   # previous column's chunks (PV still to run)
            epi_q = []  # chunks whose PV is emitted, epilogue pending
            for h in range(HPC):
                qt_b, kt_b, v_b = emit_loads(nc, pools, aps, h)
                for col in range(NCOL):
                    p_col = p_pool.tile([P, KT * CW], bf16, tag="p")
                    last_col = h == HPC - 1 and col == NCOL - 1

                    def weave_pv(n):
                        left = n
                        for c in prev:
                            left -= c.step(nc, pools, aps, left)
                            if left == 0:
                                break

                    for si, (eng, kt0, n) in enumerate(SEGS):
                        for kt_i in range(kt0, kt0 + n):
                            emit_qk(nc, s_arena, qt_b, kt_b, kt_i, col)
                        emit_exp_seg(nc, s_arena, p_col, eng, kt0, n)
                        # deferred epilogues of the column before `prev`:
                        # deps are a column old, so no FIFO blocking.
                        if si == 5 and epi_q:
                            epi_q[0].epi_a(nc, pools)
                        if si == 7 and epi_q:
                            epi_q.pop(0).epi_b(nc, pools, aps)
                        if si == 8 and epi_q:
                            epi_q[0].epi_a(nc, pools)
                        if si == 10 and epi_q:
                            epi_q.pop(0).epi_b(nc, pools, aps)
                        if prev and PV_AFTER[si]:
                            weave_pv(PV_AFTER[si])
                    for c in prev:
                        c.step(nc, pools, aps, KT)  # remainder (no-op)
                        assert c.done()
                        epi_q.append(c)

                    cur = [
                        PVChunk(h, 2 * col + s, p_col, v_b) for s in range(2)
                    ]
                    if last_col:
                        for c in cur:
                            c.step(nc, pools, aps, KT)
                            epi_q.append(c)
                    prev = cur

            # drain epilogues (up to 4 chunks at the end)
            for c in epi_q:
                if not c.a_done:
                    c.epi_a(nc, pools)
            for c in epi_q:
                if not c.b_done:
                    c.epi_b(nc, pools, aps)

    nc.compile()
    return nc


def shard_inputs(Q, K, V):
    Qh = np.asarray(Q, dtype=np.float32).reshape(B * H, S, D)
    Kh = np.asarray(K, dtype=np.float32).reshape(B * H, S, D)
    Vh = np.asarray(V, dtype=np.float32).reshape(B * H, S, D)

    in_maps = []
    for c in range(NCORES):
        sl = slice(c * HPC, (c + 1) * HPC)
        qt = np.empty((HPC, P, S), dtype=ml_dtypes.bfloat16)
        kt = np.empty((HPC, P, S), dtype=ml_dtypes.bfloat16)
        qt[:, :D, :] = Qh[sl].transpose(0, 2, 1).astype(ml_dtypes.bfloat16)
        kt[:, :D, :] = Kh[sl].transpose(0, 2, 1).astype(ml_dtypes.bfloat16)
        qt[:, D:, :] = qt[:, :D, :]
        kt[:, D:, :] = kt[:, :D, :]
        vp = np.ones((HPC, S, D + 1), dtype=np.float32)
        vp[:, :, :D] = Vh[sl]
        vp = (
            vp.reshape(HPC, KT, P, D + 1)
            .transpose(0, 2, 1, 3)
            .astype(ml_dtypes.bfloat16)
        )
        in_maps.append({"qt": np.ascontiguousarray(qt),
                        "kt": np.ascontiguousarray(kt),
                        "vp": np.ascontiguousarray(vp)})
    return in_maps


_NC_CACHE = None


def kernel(Q, K, V):
    global _NC_CACHE
    if _NC_CACHE is None:
        _NC_CACHE = build_nc()
    nc = _NC_CACHE
    in_maps = shard_inputs(Q, K, V)
    res = run_bass_kernel_spmd(nc, in_maps, core_ids=list(range(NCORES)))
    out = np.empty((B * H, S, D), dtype=np.float32)
    for c in range(NCORES):
        out[c * HPC : (c + 1) * HPC] = res.results[c]["ot"].transpose(0, 2, 1)
    return out.reshape(B, H, S, D)


if __name__ == "__main__":
    nc = build_nc()
    print("compiled OK")
